# revision 18
# baseline (speedup 1.0000x reference)
import sys

import numpy as np

N, D, H, E = 20000, 256, 8, 320000
C = D // H
GAT_SLOPE = 0.2
SEM_SLOPE = 0.01
NHALF = 10000
NH = 10112
NT = 20224
QW = 2528
ROW = 384
ALROW = 128

PERM = np.array([(f % 8) * 32 + f // 8 for f in range(D)], dtype=np.int64)


def _leaky(x, slope):
    return np.where(x >= 0, x, slope * x).astype(np.float32)


def _gat_np(x_s, x_t, src, dst, Ws, Wd, a_s, a_d, b):
    n = x_t.shape[0]
    hs = (x_s @ Ws).reshape(n, H, C)
    hd = (x_t @ Wd).reshape(n, H, C)
    al_s = (hs * a_s).sum(-1)
    al_d = (hd * a_d).sum(-1)
    keep = src != dst
    loop = np.arange(n, dtype=src.dtype)
    src_f = np.concatenate([src, loop])
    dst_f = np.concatenate([dst, loop])
    mask = np.concatenate([keep, np.ones(n, bool)])
    alpha = _leaky(al_s[src_f] + al_d[dst_f], GAT_SLOPE)
    order = np.argsort(dst_f, kind="stable")
    starts = np.searchsorted(dst_f[order], np.arange(n, dtype=np.int64))
    m = np.maximum.reduceat(alpha[order], starts, axis=0)
    ex = np.exp(alpha - m[dst_f])
    ex[~mask] = 0.0
    den = np.add.reduceat(ex[order], starts, axis=0)
    attn = ex / den[dst_f]
    msg = (attn[:, :, None] * hs[src_f]).reshape(-1, D)
    out = np.add.reduceat(msg[order], starts, axis=0)
    return np.maximum(out + b, 0.0).astype(np.float32)


def _kernel_numpy(x_src, x_node, edges, ew, W_src, W_dst, att_src, att_dst,
                  bias, u, lin_W, lin_b):
    aggs = []
    for r in range(4):
        src, dst = edges[r, 1], edges[r, 0]
        aggs.append(_gat_np(x_src[r], x_node, src, dst, W_src[r], W_dst[r],
                            att_src[r], att_dst[r], bias[r]))
    u_a, u_x = u[:D], u[D:]
    xu = x_node @ u_x
    scores = [np.exp(_leaky(a @ u_a + xu, SEM_SLOPE)) for a in aggs]
    ssum = scores[0] + scores[1] + scores[2] + scores[3]
    combined = np.zeros_like(aggs[0])
    for s, a in zip(scores, aggs):
        combined += (s / ssum) * a
    W_x, W_c = lin_W[:D], lin_W[D:]
    h = np.maximum(x_node @ W_x + combined @ W_c + lin_b, 0.0)
    norm = np.maximum(np.linalg.norm(h, axis=-1, keepdims=True), 1e-12)
    return (h / norm).astype(np.float32)


def _wrap_idx(flat_i16):
    a = flat_i16.reshape(-1, 16).T
    return np.tile(a, (8, 1)).copy()


def _host_prep(x_src, x_node, edges, W_src, W_dst, att_src, att_dst, u, lin_W):
    import ml_dtypes
    bf16 = ml_dtypes.bfloat16
    per_core = []
    nch = np.zeros((8, 79), np.int64)
    core_ed = []
    for c in range(8):
        r, Hh = c % 4, c // 4
        src = edges[r, 1].astype(np.int64)
        dst = edges[r, 0].astype(np.int64)
        lo = NHALF * Hh
        m = (src != dst) & (dst >= lo) & (dst < lo + NHALF)
        src = src[m]
        dstl = dst[m] - lo
        loop = np.arange(NHALF, dtype=np.int64)
        src = np.concatenate([src, loop + lo])
        dstl = np.concatenate([dstl, loop])
        o = np.argsort(dstl, kind="stable")
        src, dstl = src[o], dstl[o]
        t = dstl >> 7
        cnt = np.bincount(t, minlength=79)
        nch[c] = (cnt + 127) >> 7
        core_ed.append((src, dstl, np.cumsum(cnt) - cnt, cnt))
    K = nch.max(axis=0)
    NCH = int(K.sum())
    off = np.cumsum(K) - K
    for c in range(8):
        src, dstl, starts, cnt = core_ed[c]
        g1 = np.zeros(NCH * 128, np.int64)
        g2 = np.zeros(NCH * 128, np.int64)
        dr = np.full(NCH * 128, 999.0, np.float32)
        for t in range(79):
            s0, n0 = starts[t], cnt[t]
            p0 = off[t] * 128
            g1[p0:p0 + n0] = src[s0:s0 + n0]
            g2[p0:p0 + n0] = dstl[s0:s0 + n0]
            dr[p0:p0 + n0] = (dstl[s0:s0 + n0] - 128 * t).astype(np.float32)
        per_core.append({
            "g1i": _wrap_idx(g1.astype(np.int16)),
            "g2i": _wrap_idx(g2.astype(np.int16)),
            "dr": dr.reshape(NCH, 128).T.copy(),
        })
    in_maps = []
    for c in range(8):
        r, Hh = c % 4, c // 4
        q = c % 4
        lo = NHALF * Hh
        A_s = np.zeros((D, H), np.float32)
        A_d = np.zeros((D, H), np.float32)
        for h in range(H):
            A_s[h * C:(h + 1) * C, h] = att_src[r, h]
            A_d[h * C:(h + 1) * C, h] = att_dst[r, h]
        xsT = np.zeros((D, NT), bf16)
        xsT[:, :N] = x_src[r].T.astype(bf16)
        xnT_half = np.zeros((D, NH), bf16)
        xnT_half[:, :NHALF] = x_node[lo:lo + NHALF].T.astype(bf16)
        q0 = QW * q
        q1 = min(q0 + QW, NHALF)
        xnT_q = np.zeros((D, QW), bf16)
        xnT_q[:, :q1 - q0] = x_node[lo + q0:lo + q1].T.astype(bf16)
        uu = np.concatenate([u[:D][PERM], u[D:]]).astype(np.float32)
        linW = np.concatenate([lin_W[:D], lin_W[D:][PERM]]).astype(np.float32)
        iotam = np.tile(np.arange(128, dtype=np.float32), (128, 1)).copy()
        in_maps.append({
            "xsT": xsT, "xnT_half": xnT_half, "xnT_q": xnT_q,
            "Wp": np.ascontiguousarray(W_src[r][:, PERM]).astype(np.float32),
            "WsT": np.ascontiguousarray(W_src[r].T).astype(np.float32),
            "WdT": np.ascontiguousarray(W_dst[r].T).astype(np.float32),
            "A_s": A_s, "A_d": A_d, "uu": uu.reshape(512, 1),
            "linW": linW, "iotam": iotam,
            "g1i": per_core[c]["g1i"], "g2i": per_core[c]["g2i"],
            "dr": per_core[c]["dr"],
        })
    return in_maps, K, off, NCH


def _build_graph(K, off, NCH):
    import concourse.bass as bass
    import concourse.mybir as mybir
    import concourse.tile as tile
    from concourse.masks import make_identity
    from contextlib import ExitStack

    BF = mybir.dt.bfloat16
    F32 = mybir.dt.float32
    I16 = mybir.dt.int16
    AF = mybir.ActivationFunctionType
    OP = mybir.AluOpType
    KMAX = int(K.max())

    nc = bass.Bass()
    p_in = {}
    for name, shape, dt in [
        ("xsT", [D, NT], BF), ("xnT_half", [D, NH], BF), ("xnT_q", [D, QW], BF),
        ("Wp", [D, D], F32), ("WsT", [D, D], F32), ("WdT", [D, D], F32),
        ("A_s", [D, H], F32), ("A_d", [D, H], F32), ("uu", [512, 1], F32),
        ("linW", [512, D], F32), ("iotam", [128, 128], F32),
        ("g1i", [128, NCH * 8], I16), ("g2i", [128, NCH * 8], I16),
        ("dr", [128, NCH], F32),
    ]:
        p_in[name] = nc.declare_dram_parameter(name, shape, dt, isOutput=False)
    out_hT = nc.declare_dram_parameter("out_hT", [D, QW], F32, isOutput=True)

    table = nc.dram_tensor("table", [NT, ROW], BF)
    aldram = nc.dram_tensor("aldram", [NH, ALROW], BF)
    arbuf = nc.dram_tensor("arbuf", [4 * 257, QW], BF)
    arout = nc.dram_tensor("arout", [257, QW], BF)

    es = ExitStack()
    ccsem = es.enter_context(nc.semaphore("ccsem"))
    dwsem = es.enter_context(nc.semaphore("dwsem"))

    with tile.TileContext(nc) as tc:
        with es:
            pcon = es.enter_context(tc.tile_pool(name="pcon", bufs=1))
            iota_sb = pcon.tile([128, 128], BF)
            nc.gpsimd.dma_start(out=iota_sb[:], in_=p_in["iotam"][:])
            ident = pcon.tile([128, 128], BF)
            make_identity(nc, ident[:])
            ones_sb = pcon.tile([128, 1], BF)
            nc.vector.memset(ones_sb[:], 1.0)
            eps_sb = pcon.tile([1, 1], F32)
            nc.vector.memset(eps_sb[:], 1e-24)
            g1sb = pcon.tile([128, NCH * 8], I16)
            nc.sync.dma_start(out=g1sb[:], in_=p_in["g1i"][:])
            g2sb = pcon.tile([128, NCH * 8], I16)
            nc.sync.dma_start(out=g2sb[:], in_=p_in["g2i"][:])
            drsb = pcon.tile([128, NCH], F32)
            nc.sync.dma_start(out=drsb[:], in_=p_in["dr"][:])
            WsT_sb = [pcon.tile([128, D], BF, tag="WsT", name=f"WsT{k}") for k in range(2)]
            WdT_sb = [pcon.tile([128, D], BF, tag="WdT", name=f"WdT{k}") for k in range(2)]
            As_sb = [pcon.tile([128, H], BF, tag="As", name=f"As{k}") for k in range(2)]
            Ad_sb = [pcon.tile([128, H], BF, tag="Ad", name=f"Ad{k}") for k in range(2)]
            for k in range(2):
                sl = slice(k * 128, (k + 1) * 128)
                nc.gpsimd.dma_start(out=WsT_sb[k][:], in_=p_in["WsT"][sl, :])
                nc.gpsimd.dma_start(out=WdT_sb[k][:], in_=p_in["WdT"][sl, :])
                nc.gpsimd.dma_start(out=As_sb[k][:], in_=p_in["A_s"][sl, :])
                nc.gpsimd.dma_start(out=Ad_sb[k][:], in_=p_in["A_d"][sl, :])
            Wcat = [pcon.tile([128, D + 16], BF, tag="Wcat", name=f"Wcat{k}") for k in range(2)]
            with tc.tile_pool(name="pfold", bufs=1, space="PSUM") as pf:
                for dh in range(2):
                    pfs = pf.tile([128, 8], F32, tag="fs")
                    pfd = pf.tile([128, 8], F32, tag="fd")
                    dsl = slice(dh * 128, (dh + 1) * 128)
                    for kh in range(2):
                        nc.tensor.matmul(pfs[:], lhsT=WsT_sb[kh][:, dsl],
                                         rhs=As_sb[kh][:], start=kh == 0, stop=kh == 1)
                        nc.tensor.matmul(pfd[:], lhsT=WdT_sb[kh][:, dsl],
                                         rhs=Ad_sb[kh][:], start=kh == 0, stop=kh == 1)
                    nc.gpsimd.dma_start(out=Wcat[dh][:, 0:D], in_=p_in["Wp"][dsl, :])
                    nc.scalar.activation(out=Wcat[dh][:, D:D + 8], in_=pfs[:], func=AF.Copy)
                    nc.scalar.activation(out=Wcat[dh][:, D + 8:D + 16], in_=pfd[:], func=AF.Copy)

            with tc.tile_pool(name="p1", bufs=3) as p1, \
                 tc.tile_pool(name="p1p", bufs=2, space="PSUM") as p1p:
                g0 = 0
                while g0 < NT:
                    gw = min(1024, NT - g0)
                    xs0 = p1.tile([128, 1024], BF, tag="xs0")
                    xs1 = p1.tile([128, 1024], BF, tag="xs1")
                    nc.sync.dma_start(out=xs0[:, 0:gw], in_=p_in["xsT"][0:128, g0:g0 + gw])
                    nc.sync.dma_start(out=xs1[:, 0:gw], in_=p_in["xsT"][128:256, g0:g0 + gw])
                    for i in range(gw // 128):
                        isl = slice(i * 128, (i + 1) * 128)
                        px = p1p.tile([128, D + 8], F32, tag="px")
                        nc.tensor.matmul(px[:], lhsT=xs0[:, isl], rhs=Wcat[0][:, 0:D + 8],
                                         start=True, stop=False)
                        nc.tensor.matmul(px[:], lhsT=xs1[:, isl], rhs=Wcat[1][:, 0:D + 8],
                                         start=False, stop=True)
                        tb = p1.tile([128, D + 8], BF, tag="tb")
                        nc.scalar.activation(out=tb[:], in_=px[:], func=AF.Copy)
                        nc.sync.dma_start(out=table[g0 + i * 128:g0 + (i + 1) * 128, 0:D + 8],
                                          in_=tb[:])
                    g0 += gw

            with tc.tile_pool(name="p2", bufs=3) as p2, \
                 tc.tile_pool(name="p2p", bufs=2, space="PSUM") as p2p:
                g0 = 0
                while g0 < NH:
                    gw = min(1024, NH - g0)
                    xn0 = p2.tile([128, 1024], BF, tag="xn0")
                    xn1 = p2.tile([128, 1024], BF, tag="xn1")
                    nc.sync.dma_start(out=xn0[:, 0:gw], in_=p_in["xnT_half"][0:128, g0:g0 + gw])
                    nc.sync.dma_start(out=xn1[:, 0:gw], in_=p_in["xnT_half"][128:256, g0:g0 + gw])
                    for i in range(gw // 128):
                        isl = slice(i * 128, (i + 1) * 128)
                        pa = p2p.tile([128, 8], F32, tag="pa")
                        nc.tensor.matmul(pa[:], lhsT=xn0[:, isl], rhs=Wcat[0][:, D + 8:D + 16],
                                         start=True, stop=False)
                        nc.tensor.matmul(pa[:], lhsT=xn1[:, isl], rhs=Wcat[1][:, D + 8:D + 16],
                                         start=False, stop=True)
                        ald = p2.tile([128, 8], BF, tag="ald")
                        nc.scalar.activation(out=ald[:], in_=pa[:], func=AF.Copy)
                        nc.sync.dma_start(out=aldram[g0 + i * 128:g0 + (i + 1) * 128, 0:8],
                                          in_=ald[:])
                    g0 += gw

            aggT = [pcon.tile([128, NH], BF, tag="aggT", name=f"aggT{k}") for k in range(2)]
            gidx_reg = nc.alloc_register(mybir.EngineType.Pool, "gidxreg")
            with tc.tile_pool(name="p3", bufs=2) as p3, \
                 tc.tile_pool(name="p3s", bufs=3) as p3s, \
                 tc.tile_pool(name="p3p", bufs=2, space="PSUM") as p3p, \
                 tc.tile_pool(name="p3t", bufs=2, space="PSUM") as p3t:
                for t in range(79):
                    Kt = int(K[t])
                    b16 = int(off[t]) * 8
                    bc = int(off[t])
                    mb = p3.tile([128, KMAX, ROW], BF, tag="mb")
                    nc.gpsimd.reg_mov(gidx_reg, Kt * 128)
                    nc.gpsimd.dma_gather(
                        mb[:, 0:Kt, :], table[:, :], g1sb[:, b16:b16 + Kt * 8],
                        Kt * 128, gidx_reg, ROW)
                    ab = p3.tile([128, KMAX, ALROW], BF, tag="ab")
                    nc.gpsimd.dma_gather(
                        ab[:, 0:Kt, :], aldram[:, :], g2sb[:, b16:b16 + Kt * 8],
                        Kt * 128, gidx_reg, ALROW)
                    exs = p3.tile([128, KMAX, 8], BF, tag="exs")
                    nc.vector.tensor_tensor(
                        out=exs[:, 0:Kt, :], in0=mb[:, 0:Kt, D:D + 8],
                        in1=ab[:, 0:Kt, 0:8], op=OP.add)
                    nc.scalar.activation(out=exs[:, 0:Kt, :], in_=exs[:, 0:Kt, :],
                                         func=AF.Lrelu, alpha=GAT_SLOPE)
                    nc.scalar.activation(out=exs[:, 0:Kt, :], in_=exs[:, 0:Kt, :],
                                         func=AF.Exp)
                    exb = exs[:, 0:Kt, :].rearrange("p k h -> p k () h").broadcast_to(
                        [128, Kt, 32, 8])
                    nc.vector.tensor_tensor(
                        out=mb[:, 0:Kt, 0:D].rearrange("p k (j h) -> p k j h", h=8),
                        in0=mb[:, 0:Kt, 0:D].rearrange("p k (j h) -> p k j h", h=8),
                        in1=exb, op=OP.mult)
                    nc.vector.tensor_copy(out=mb[:, 0:Kt, D:D + 8], in_=exs[:, 0:Kt, :])
                    pagg = p3p.tile([128, D + 8], F32, tag="pagg")
                    for cc in range(Kt):
                        sT = p3s.tile([128, 128], BF, tag="sT")
                        nc.vector.tensor_scalar(
                            out=sT[:], in0=iota_sb[:], scalar1=drsb[:, bc + cc:bc + cc + 1],
                            scalar2=None, op0=OP.is_equal)
                        nc.tensor.matmul(pagg[:], lhsT=sT[:], rhs=mb[:, cc, 0:D + 8],
                                         start=cc == 0, stop=cc == Kt - 1)
                    rc = p3s.tile([128, 8], F32, tag="rc")
                    nc.vector.reciprocal(out=rc[:], in_=pagg[:, D:D + 8])
                    agg = p3s.tile([128, D], BF, tag="agg")
                    rcb = rc[:].rearrange("p h -> p () h").broadcast_to([128, 32, 8])
                    nc.vector.tensor_tensor(
                        out=agg[:].rearrange("p (j h) -> p j h", h=8),
                        in0=pagg[:, 0:D].rearrange("p (j h) -> p j h", h=8),
                        in1=rcb, op=OP.mult)
                    nc.scalar.activation(out=agg[:], in_=agg[:], func=AF.Relu)
                    for fh in range(2):
                        ptr = p3t.tile([128, 128], BF, tag="ptr")
                        nc.tensor.transpose(out=ptr[:], in_=agg[:, fh * 128:(fh + 1) * 128],
                                            identity=ident[:])
                        nc.scalar.activation(out=aggT[fh][:, t * 128:(t + 1) * 128],
                                             in_=ptr[:], func=AF.Copy)

            srow = pcon.tile([1, NH], BF)
            uu_sb = [pcon.tile([128, 1], BF, tag="uu", name=f"uu{k}") for k in range(4)]
            for it in range(4):
                nc.gpsimd.dma_start(out=uu_sb[it][:], in_=p_in["uu"][it * 128:(it + 1) * 128, :])
            with tc.tile_pool(name="p4", bufs=3) as p4, \
                 tc.tile_pool(name="p4p", bufs=2, space="PSUM") as p4p:
                sprev = pcon.tile([1, NH], F32)
                nt_sizes = []
                pos = 0
                while pos < NH:
                    w = min(512, NH - pos)
                    nt_sizes.append((pos, w))
                    pos += w
                for (pos, w) in nt_sizes:
                    ps = p4p.tile([1, 512], F32, tag="ps")
                    for fh in range(2):
                        nc.tensor.matmul(ps[:, 0:w], lhsT=uu_sb[fh][:], rhs=aggT[fh][:, pos:pos + w],
                                         start=fh == 0, stop=False)
                    for fh in range(2):
                        xn = p4.tile([128, 512], BF, tag="xn4")
                        nc.sync.dma_start(out=xn[:, 0:w],
                                          in_=p_in["xnT_half"][fh * 128:(fh + 1) * 128, pos:pos + w])
                        nc.tensor.matmul(ps[:, 0:w], lhsT=uu_sb[2 + fh][:], rhs=xn[:, 0:w],
                                         start=False, stop=fh == 1)
                    nc.scalar.activation(out=sprev[:, pos:pos + w], in_=ps[:, 0:w], func=AF.Copy)
                nc.scalar.activation(out=sprev[:], in_=sprev[:], func=AF.Lrelu, alpha=SEM_SLOPE)
                nc.scalar.activation(out=srow[:], in_=sprev[:], func=AF.Exp)
                for fh in range(2):
                    nc.vector.tensor_tensor(out=aggT[fh][:], in0=aggT[fh][:],
                                            in1=srow[:].broadcast_to([128, NH]), op=OP.mult)

            with tc.tile_critical():
                nd = 0
                for q in range(4):
                    qs = slice(q * QW, (q + 1) * QW)
                    for fh in range(2):
                        nc.sync.dma_start(out=arbuf[q * 257 + fh * 128:q * 257 + (fh + 1) * 128, :],
                                          in_=aggT[fh][:, qs]).then_inc(dwsem, 16)
                        nd += 1
                    nc.sync.dma_start(out=arbuf[q * 257 + 256:q * 257 + 257, :],
                                      in_=srow[:, qs]).then_inc(dwsem, 16)
                    nd += 1
                nc.gpsimd.wait_ge(dwsem, nd * 16)
                nc.gpsimd.collective_compute(
                    "ReduceScatter", mybir.AluOpType.add,
                    replica_groups=[[0, 1, 2, 3], [4, 5, 6, 7]],
                    ins=[arbuf[:]], outs=[arout[:]],
                ).then_inc(ccsem)
                nc.gpsimd.wait_ge(ccsem, 1)

            with tc.tile_pool(name="p6", bufs=1) as p6, \
                 tc.tile_pool(name="p6p", bufs=2, space="PSUM") as p6p:
                numT = [p6.tile([128, QW], BF, tag="numT", name=f"numT{i}") for i in range(2)]
                for fh in range(2):
                    nc.sync.dma_start(out=numT[fh][:], in_=arout[fh * 128:(fh + 1) * 128, :])
                ssum = p6.tile([1, QW], BF, tag="ssum")
                nc.sync.dma_start(out=ssum[:], in_=arout[256:257, :])
                rs = p6.tile([1, QW], F32, tag="rs")
                nc.vector.reciprocal(out=rs[:], in_=ssum[:])
                rsb = p6.tile([1, QW], BF, tag="rsb")
                nc.vector.tensor_copy(out=rsb[:], in_=rs[:])
                for fh in range(2):
                    nc.vector.tensor_tensor(out=numT[fh][:], in0=numT[fh][:],
                                            in1=rsb[:].broadcast_to([128, QW]), op=OP.mult)
                xnq = [p6.tile([128, QW], BF, tag="xnq", name=f"xnq{i}") for i in range(2)]
                for fh in range(2):
                    nc.sync.dma_start(out=xnq[fh][:], in_=p_in["xnT_q"][fh * 128:(fh + 1) * 128, :])
                lw = [p6.tile([128, D], BF, tag="lw", name=f"lw{i}") for i in range(4)]
                for it in range(4):
                    nc.gpsimd.dma_start(out=lw[it][:], in_=p_in["linW"][it * 128:(it + 1) * 128, :])
                hT = [p6.tile([128, QW], BF, tag="hT", name=f"hT{i}") for i in range(2)]
                rhs4 = [xnq[0], xnq[1], numT[0], numT[1]]
                pos = 0
                while pos < QW:
                    w = min(512, QW - pos)
                    for ot in range(2):
                        ph = p6p.tile([128, 512], F32, tag="ph")
                        for it in range(4):
                            nc.tensor.matmul(ph[:, 0:w], lhsT=lw[it][:, ot * 128:(ot + 1) * 128],
                                             rhs=rhs4[it][:, pos:pos + w],
                                             start=it == 0, stop=it == 3)
                        nc.scalar.activation(out=hT[ot][:, pos:pos + w], in_=ph[:, 0:w],
                                             func=AF.Relu)
                    pn = p6p.tile([1, 512], F32, tag="pn")
                    sq = p6.tile([128, 512], BF, tag="sq", bufs=2)
                    for ot in range(2):
                        nc.vector.tensor_tensor(out=sq[:, 0:w], in0=hT[ot][:, pos:pos + w],
                                                in1=hT[ot][:, pos:pos + w], op=OP.mult)
                        nc.tensor.matmul(pn[:, 0:w], lhsT=ones_sb[:], rhs=sq[:, 0:w],
                                         start=ot == 0, stop=ot == 1)
                    sn = p6.tile([1, 512], F32, tag="sn", bufs=2)
                    nc.scalar.activation(out=sn[:, 0:w], in_=pn[:, 0:w], func=AF.Sqrt,
                                         bias=eps_sb[:])
                    rnf = p6.tile([1, 512], F32, tag="rnf", bufs=2)
                    nc.vector.reciprocal(out=rnf[:, 0:w], in_=sn[:, 0:w])
                    rn = p6.tile([1, 512], BF, tag="rn", bufs=2)
                    nc.vector.tensor_copy(out=rn[:, 0:w], in_=rnf[:, 0:w])
                    for ot in range(2):
                        of = p6.tile([128, 512], F32, tag="of", bufs=3)
                        nc.vector.tensor_tensor(out=of[:, 0:w], in0=hT[ot][:, pos:pos + w],
                                                in1=rn[:, 0:w].broadcast_to([128, w]), op=OP.mult)
                        nc.sync.dma_start(out=out_hT[ot * 128:(ot + 1) * 128, pos:pos + w],
                                          in_=of[:, 0:w])
                    pos += w
    return nc


_BASS_CACHE = {}
_EXEC_CACHE = {}
LAST_EXEC_NS = None
def _kernel_bass(x_src, x_node, edges, ew, W_src, W_dst, att_src, att_dst,
                 bias, u, lin_W, lin_b):
    sys.path.insert(0, "/opt/trn_rl_repo")
    from concourse.bass_utils import run_bass_kernel_spmd

    in_maps, K, off, NCH = _host_prep(x_src, x_node, edges, W_src, W_dst,
                                      att_src, att_dst, u, lin_W)
    key = (tuple(K.tolist()),)
    if key not in _BASS_CACHE:
        _BASS_CACHE[key] = _build_graph(K, off, NCH)
    nc = _BASS_CACHE[key]
    res = run_bass_kernel_spmd(nc, in_maps, core_ids=list(range(8)))
    out = np.zeros((N, D), np.float32)
    for c in range(8):
        Hh, q = c // 4, c % 4
        lo = NHALF * Hh
        q0, q1 = QW * q, min(QW * (q + 1), NHALF)
        out[lo + q0:lo + q1] = res.results[c]["out_hT"].T[0:q1 - q0]
    return out


def kernel(x_src, x_node, edges, ew, W_src, W_dst, att_src, att_dst,
           bias, u, lin_W, lin_b):
    args = [np.asarray(a) for a in (x_src, x_node, edges, ew, W_src, W_dst,
                                    att_src, att_dst, bias, u, lin_W, lin_b)]
    x_src, x_node, edges, ew, W_src, W_dst, att_src, att_dst, bias, u, lin_W, lin_b = args
    if np.abs(bias).max() <= 1e-20 and np.abs(lin_b).max() <= 1e-20:
        for attempt in range(2):
            try:
                return _kernel_bass(x_src, x_node, edges, ew, W_src, W_dst,
                                    att_src, att_dst, bias, u, lin_W, lin_b)
            except Exception as e:
                print(f"[kernel] bass attempt {attempt} failed "
                      f"({type(e).__name__}: {e})", file=sys.stderr)
                _BASS_CACHE.clear()
    return _kernel_numpy(x_src, x_node, edges, ew, W_src, W_dst,
                         att_src, att_dst, bias, u, lin_W, lin_b)
